# revision 14
# baseline (speedup 1.0000x reference)
"""LSTM encoder (T=512, B=256, H=256, V=32000) on 8 trn2 NeuronCores.

Strategy
--------
Data-parallel over batch: B=256 -> 32 per core; weights/table replicated.

Per core the recurrence runs in a transposed "gatesT" layout: gates live as
[4H on partitions (8 chunks of 128), batch(32) in the free dim], so the
ACT/DVE elementwise chain per step touches only 32-64 columns per op.
Weight chunks are the stationary matmul operand (fp16, FWL), h streams as
the 32-column moving operand.

The input projections W_ih @ emb_t (+ bias) for a window of 8 future steps
are precomputed into the window's PSUM banks by efficient 256-column
matmuls; the per-step W_hh matmuls then accumulate straight on top
(start=False), so no separate "add X" op exists in the serial chain.

Embeddings are fetched with dma_gather(transpose=True), which gathers fp16
table rows and deposits them H-on-partitions, exactly the rhs layout the
X-projection matmuls need.

Numerics: fp16 table/weights/h (matmul operands), fp32 PSUM accumulate and
fp32 elementwise state c. Host-side emulation of this scheme measures
absmax/scale error ~4.5e-4 vs the fp32 reference.

Gate chunk order is permuted host-side to [i, f, o, g] so one sigmoid call
covers i,f,o (contiguous) and one tanh covers g.
"""

import numpy as np

T, B, H, V = 512, 256, 256, 32000
N_CORES = 8
BL = B // N_CORES          # 32 batch per core
S = 8                      # steps per PSUM window
G4 = 4 * H                 # 1024
M = G4 // 128              # 8 gate chunks
K = H // 128               # 2 contraction chunks

# gate row order i, f, o, g (PyTorch native is i, f, g, o)
_PERM = np.concatenate([
    np.arange(0, H), np.arange(H, 2 * H),
    np.arange(3 * H, 4 * H), np.arange(2 * H, 3 * H),
])


def _build_bass(t_steps=T):
    from contextlib import ExitStack
    from concourse import bacc, mybir, library_config
    import concourse.tile as tile

    f16, f32, i16 = mybir.dt.float16, mybir.dt.float32, mybir.dt.int16
    Sig = mybir.ActivationFunctionType.Sigmoid
    Tanh = mybir.ActivationFunctionType.Tanh
    mult, add = mybir.AluOpType.mult, mybir.AluOpType.add

    NW = t_steps // S
    NI = S * BL            # 256 gathered rows per window

    nc = bacc.Bacc("TRN2", target_bir_lowering=False, debug=False)
    idx_d = nc.declare_dram_parameter("idx", [128, t_steps * BL // 16], i16, isOutput=False)
    tab_d = nc.declare_dram_parameter("table", [V, H], f16, isOutput=False)
    wih_d = nc.declare_dram_parameter("wih_t", [H, G4], f16, isOutput=False)
    whh_d = nc.declare_dram_parameter("whh_t", [H, G4], f16, isOutput=False)
    b_d = nc.declare_dram_parameter("bias", [2, G4 // 2], f16, isOutput=False)
    ind_d = nc.declare_dram_parameter("ind", [2, 2 * S * BL], f16, isOutput=False)
    h0_d = nc.declare_dram_parameter("h0t", [128, K, BL], f16, isOutput=False)
    c0_d = nc.declare_dram_parameter("c0t", [128, K, BL], f32, isOutput=False)
    ho_d = nc.declare_dram_parameter("h_out", [128, K, BL], f32, isOutput=True)
    co_d = nc.declare_dram_parameter("c_out", [128, K, BL], f32, isOutput=True)

    with tile.TileContext(nc) as tc, ExitStack() as ctx:
        const = ctx.enter_context(tc.tile_pool(name="const", bufs=1))
        embp = ctx.enter_context(tc.tile_pool(name="embp", bufs=4))
        psum = ctx.enter_context(tc.tile_pool(name="psum", bufs=2, space="PSUM"))
        sp = ctx.enter_context(tc.tile_pool(name="sp", bufs=2))
        tmp = ctx.enter_context(tc.tile_pool(name="tmp", bufs=2))
        hp = ctx.enter_context(tc.tile_pool(name="hp", bufs=2))

        whh_sb, wih_sb = [], []
        for k in range(K):
            wt = const.tile([128, G4], f16, name=f"whh_sb{k}")
            nc.sync.dma_start(wt[:], whh_d[128 * k:128 * (k + 1), :])
            whh_sb.append(wt)
            xt = const.tile([128, G4], f16, name=f"wih_sb{k}")
            nc.sync.dma_start(xt[:], wih_d[128 * k:128 * (k + 1), :])
            wih_sb.append(xt)
        b_sb = const.tile([2, G4 // 2], f16, name="b_sb")
        nc.sync.dma_start(b_sb[:], b_d[:])
        # indicator[j, n] = 1 where n's m-chunk parity == j (bank-bias rhs)
        ind = const.tile([2, 2 * S * BL], f16, name="ind")
        nc.sync.dma_start(ind[:], ind_d[:])
        nc.gpsimd.load_library(library_config.mlp)
        idx_sb = const.tile([128, t_steps * BL // 16], i16, name="idx_sb")
        nc.sync.dma_start(idx_sb[:], idx_d[:])
        c_t = const.tile([128, K, BL], f32, name="c_t")
        nc.sync.dma_start(c_t[:], c0_d[:])
        h0 = const.tile([128, K, BL], f16, name="h0_sb")
        nc.sync.dma_start(h0[:], h0_d[:])
        h_cur = h0

        for w in range(NW):
            embt = embp.tile([128, K, NI], f16, name="embt", tag="embt")
            nc.gpsimd.dma_gather(
                out_ap=embt[:], in_ap=tab_d[:],
                idxs_ap=idx_sb[:, (NI // 16) * w:(NI // 16) * (w + 1)],
                num_idxs=NI, num_idxs_reg=NI, elem_size=H, transpose=True)
            ps = psum.tile([128, M, S, BL], f32, name="ps", tag="ps")
            # One PSUM bank holds two m-chunks (2x256 f32 cols) and start=True
            # zeroes the whole 2KB bank. So the FIRST write to each bank is a
            # single full-bank (N=512) bias matmul with start=True: lhsT packs
            # the bank's two bias chunks as K=2, rhs is a 0/1 indicator
            # selecting which half each row lands in. All X / recurrent
            # matmuls then accumulate (start=False) and are WAW-ordered
            # against the bank write by Tile.
            for b in range(M // 2):
                nc.tensor.matmul(
                    out=ps[:, 2 * b:2 * b + 2, :, :],
                    lhsT=b_sb[:, 128 * b:128 * (b + 1)],
                    rhs=ind[:], start=True, stop=False, skip_group_check=True)
            for m in range(M):
                for k in range(K):
                    nc.tensor.matmul(
                        out=ps[:, m, :, :], lhsT=wih_sb[k][:, 128 * m:128 * (m + 1)],
                        rhs=embt[:, k, :], start=False, stop=False,
                        skip_group_check=True)
            for s in range(S):
                t = w * S + s
                for m in range(M):
                    for k in range(K):
                        nc.tensor.matmul(
                            out=ps[:, m, s, :],
                            lhsT=whh_sb[k][:, 128 * m:128 * (m + 1)],
                            rhs=h_cur[:, k, :],
                            start=False, stop=(k == K - 1), skip_group_check=True)
                sall = sp.tile([128, M, BL], f32, name="sall", tag="sall")
                nc.scalar.activation(sall[:, 0:6, :], ps[:, 0:6, s, :], Sig)
                nc.scalar.activation(sall[:, 6:8, :], ps[:, 6:8, s, :], Tanh)
                m1 = tmp.tile([128, K, BL], f32, name="m1", tag="m1")
                nc.vector.tensor_tensor(m1[:], sall[:, 2:4, :], c_t[:], mult)
                m2 = tmp.tile([128, K, BL], f32, name="m2", tag="m2")
                nc.vector.tensor_tensor(m2[:], sall[:, 0:2, :], sall[:, 6:8, :], mult)
                nc.vector.tensor_tensor(c_t[:], m1[:], m2[:], add)
                tct = tmp.tile([128, K, BL], f32, name="tct", tag="tct")
                nc.scalar.activation(tct[:], c_t[:], Tanh)
                if t < t_steps - 1:
                    hn = hp.tile([128, K, BL], f16, name="hn", tag="hn")
                    nc.vector.tensor_tensor(hn[:], sall[:, 4:6, :], tct[:], mult)
                    h_cur = hn
                else:
                    hf = tmp.tile([128, K, BL], f32, name="hf", tag="hf")
                    nc.vector.tensor_tensor(hf[:], sall[:, 4:6, :], tct[:], mult)
                    nc.sync.dma_start(ho_d[:], hf[:])
                    nc.sync.dma_start(co_d[:], c_t[:])
    # Bacc register allocation etc. must run before BIR serialization.
    nc.finalize()
    return nc


def _prep_inputs(enc_inputs, h0, c0, embed, W_ih, W_hh, b_ih, b_hh, t_steps=T):
    """Host-side shard + layout prep. Returns list of per-core in_maps."""
    wih_t = np.ascontiguousarray(W_ih[_PERM].T).astype(np.float16)   # [H, 4H]
    whh_t = np.ascontiguousarray(W_hh[_PERM].T).astype(np.float16)
    # bias packed per PSUM bank: bias2[j, 128*b + p] = bias[128*(2b+j) + p]
    bias = np.ascontiguousarray(
        (b_ih + b_hh)[_PERM].astype(np.float16)
        .reshape(4, 2, 128).transpose(1, 0, 2).reshape(2, G4 // 2))
    table = embed.astype(np.float16)                                 # [V, H]
    ind = np.zeros((2, 2 * S * BL), np.float16)
    ind[0, :S * BL] = 1.0
    ind[1, S * BL:] = 1.0

    in_maps = []
    for c in range(N_CORES):
        bs = slice(c * BL, (c + 1) * BL)
        flat = enc_inputs[:t_steps, bs].astype(np.int16).reshape(-1)  # t-major
        wrapped = flat.reshape(-1, 16).T                              # [16, n/16]
        wrapped = np.ascontiguousarray(np.tile(wrapped, (8, 1)))      # [128, n/16]
        h0t = np.empty((128, K, BL), np.float16)
        c0t = np.empty((128, K, BL), np.float32)
        for k in range(K):
            h0t[:, k, :] = h0[bs].T[128 * k:128 * (k + 1), :]
            c0t[:, k, :] = c0[bs].T[128 * k:128 * (k + 1), :]
        in_maps.append({
            "idx": wrapped, "table": table, "wih_t": wih_t, "whh_t": whh_t,
            "bias": bias, "ind": ind, "h0t": h0t, "c0t": c0t,
        })
    return in_maps


def _unshard(results):
    h = np.empty((B, H), np.float32)
    c = np.empty((B, H), np.float32)
    for core, out in enumerate(results):
        bs = slice(core * BL, (core + 1) * BL)
        for k in range(K):
            h[bs, 128 * k:128 * (k + 1)] = out["h_out"][:, k, :].T
            c[bs, 128 * k:128 * (k + 1)] = out["c_out"][:, k, :].T
    return h, c


def kernel(enc_inputs, h0, c0, embed, W_ih, W_hh, b_ih, b_hh):
    from concourse.bass_utils import run_bass_kernel_spmd

    enc_inputs = np.asarray(enc_inputs)
    h0 = np.asarray(h0, dtype=np.float32)
    c0 = np.asarray(c0, dtype=np.float32)
    embed = np.asarray(embed, dtype=np.float32)
    W_ih = np.asarray(W_ih, dtype=np.float32)
    W_hh = np.asarray(W_hh, dtype=np.float32)
    b_ih = np.asarray(b_ih, dtype=np.float32)
    b_hh = np.asarray(b_hh, dtype=np.float32)

    nc = _build_bass()
    in_maps = _prep_inputs(enc_inputs, h0, c0, embed, W_ih, W_hh, b_ih, b_hh)
    res = run_bass_kernel_spmd(nc, in_maps, core_ids=list(range(N_CORES)))
    return _unshard(res.results)
